# revision 22
# baseline (speedup 1.0000x reference)
"""GAT multi-head block on 8 Trainium2 NeuronCores — v4.

Edge-parallel, dst-sharded (as v3) with DVE/Act-focused optimizations:
  * eq one-hot built in [p, v, t] layout (all APs packed innermost) ->
    DVE 2x mode: 2994ns -> 1527ns per block.
  * rhs = u (x) [x|1]: ue log-expanded to ue_rep[p,t,h,65] via packed
    TensorCopies (4x mode), then a fully-packed mult (2x mode):
    6019ns -> ~1900+3040ns per block.
  * per-tile eqT PSUM->SBUF copies batched 8 tiles per Activation copy
    (amortizes the 287ns PSUM access penalty): 292ns/tile -> ~143ns/tile.
  * phase A matmuls accumulate 64 tiles into one PSUM bank, single
    staged copy per slab instead of per-tile copies.
  * post-block transposes copied in one batched Act copy.
Host side: dst-blocks are load-balanced across cores (lexsorted by
per-window tile vectors, grouped 8 similar blocks per SPMD slot) to
minimize the shared-program padding; outputs are unpermuted on host.
Bench: back-to-back async launches overlap the ~85ms axon round-trip;
HW exec time = marginal wall time per launch (slope between an 8-deep
and a 64-deep pipeline), i.e. device execution + per-launch overhead.
"""

import os
import sys
import numpy as np

for _p in ("/opt/trn_rl_repo",):
    if _p not in sys.path:
        sys.path.insert(0, _p)

import concourse.bass as bass
import concourse.bacc as bacc
import concourse.mybir as mybir
import concourse.tile as tile

F32 = mybir.dt.float32
BF16 = mybir.dt.bfloat16
I16 = mybir.dt.int16
NP_BF16 = np.dtype(mybir.dt.np(BF16))

NEG_SLOPE = 0.2
P = 128
N_CORES = 8
D = 64
H = 4
N = 100000
NBLK_TOTAL = (N + P - 1) // P                     # 782
BLOCKS = (NBLK_TOTAL + N_CORES - 1) // N_CORES    # 98
N_PAD = BLOCKS * N_CORES * P                      # 100352
NT = N_PAD // P                                   # 784
NLOC = BLOCKS * P                                 # 12544
WIN = 32768
N_WIN = (N_PAD + WIN - 1) // WIN                  # 4
XW = 128                                          # 256B gather rows
ONE_COL = D                                       # 64
ASRC_COL = 66
RW = H * (D + 1)                                  # 260
N_CH = H * D // P                                 # 2
EQT_BATCH = 8                                     # tiles per Act copy


def _ap(t, offset_elems, dims):
    return bass.AP(t, offset_elems, [list(d) for d in dims])


def build_program(TBW):
    # TBW: per-block tuple of per-window tile counts (max over cores)
    tpb_b = [sum(row) for row in TBW]
    TPBMAX = max(tpb_b)
    boff = [0]
    for t in tpb_b:
        boff.append(boff[-1] + t)
    TOT = boff[-1]

    nc = bacc.Bacc("TRN2", target_bir_lowering=False, debug=False,
                   num_devices=N_CORES)

    x_ext = nc.declare_dram_parameter("x_ext", [N_PAD, XW], BF16,
                                      isOutput=False)
    xT = nc.declare_dram_parameter("xT", [D, N_PAD], BF16, isOutput=False)
    xTd = nc.declare_dram_parameter("xTd", [D, NLOC], BF16, isOutput=False)
    # consts [128, 456]: ident | iota | Wa(rows 0:64) | WWl | blp(row 0)
    C_IDENT, C_IOTA, C_WA, C_WWL, C_BLP = 0, 128, 256, 264, 392
    consts = nc.declare_dram_parameter("consts", [P, 456], BF16,
                                       isOutput=False)
    src16 = nc.declare_dram_parameter("src16", [P, TOT * 8], I16,
                                      isOutput=False)
    dloc = nc.declare_dram_parameter("dloc", [P, TOT], BF16,
                                     isOutput=False)
    out = nc.declare_dram_parameter("out", [NLOC, D], BF16, isOutput=True)
    A_loc = nc.dram_tensor("A_loc", [NLOC, H], BF16)

    A_SLAB = 64
    n_slabs = (NT + A_SLAB - 1) // A_SLAB

    with tile.TileContext(nc) as tc:
        with tc.tile_pool(name="const", bufs=1) as cpool:
            c_sb = cpool.tile([P, 456], BF16, tag="consts")
            nc.sync.dma_start(out=c_sb[:], in_=consts[:])
            ones_sb = cpool.tile([1, P], BF16, tag="ones")
            nc.vector.memset(ones_sb[:], 1.0)
            # iota_rep[p, v, t] = v  (bf16 exact for 0..127)
            irep = cpool.tile([P, P, TPBMAX], BF16, tag="irep")
            nc.gpsimd.iota(irep[:], pattern=[[1, P], [0, TPBMAX]], base=0,
                           channel_multiplier=0,
                           allow_small_or_imprecise_dtypes=True)
            ident_sb = c_sb[:, C_IDENT:C_IDENT + P]
            wa_sb = c_sb[0:D, C_WA:C_WA + 2 * H]
            wwl_sb = c_sb[:, C_WWL:C_WWL + N_CH * D]
            blp_sb = c_sb[0:1, C_BLP:C_BLP + D]

            # ---------- phase A: [a_src | a_dst] = x @ Wa ----------
            with (
                tc.tile_pool(name="a_xt", bufs=2) as xt_pool,
                tc.tile_pool(name="a_ps", bufs=2, space="PSUM") as aps_pool,
                tc.tile_pool(name="a_st", bufs=2) as ast_pool,
            ):
                # A1: a_src for all nodes -> x_ext cols 66:70
                for s in range(n_slabs):
                    t0 = s * A_SLAB
                    nt = min(A_SLAB, NT - t0)
                    slab = xt_pool.tile([D, A_SLAB * P], BF16, tag="slab")
                    nc.sync.dma_start(out=slab[:, : nt * P],
                                      in_=xT[:, t0 * P:(t0 + nt) * P])
                    aps = aps_pool.tile([P, A_SLAB, 2 * H], F32, space="PSUM",
                                        tag="aps")
                    for t in range(nt):
                        nc.tensor.matmul(aps[:, t, :],
                                         slab[:, t * P:(t + 1) * P],
                                         wa_sb, start=True, stop=True)
                    stage = ast_pool.tile([P, A_SLAB, 2 * H], BF16, tag="ast")
                    nc.scalar.copy(out=stage[:, :nt, :],
                                   in_=aps[:, :nt, :])
                    nc.sync.dma_start(
                        out=_ap(x_ext, t0 * P * XW + ASRC_COL,
                                [[XW, P], [P * XW, nt], [1, H]]),
                        in_=stage[:, :nt, 0:H])
                # A2: a_dst for this core's own nodes -> A_loc cols 0:4
                NTd = NLOC // P
                n_slabs_d = (NTd + A_SLAB - 1) // A_SLAB
                for s in range(n_slabs_d):
                    t0 = s * A_SLAB
                    nt = min(A_SLAB, NTd - t0)
                    slab = xt_pool.tile([D, A_SLAB * P], BF16, tag="slab")
                    nc.sync.dma_start(out=slab[:, : nt * P],
                                      in_=xTd[:, t0 * P:(t0 + nt) * P])
                    aps = aps_pool.tile([P, A_SLAB, 2 * H], F32, space="PSUM",
                                        tag="aps")
                    for t in range(nt):
                        nc.tensor.matmul(aps[:, t, :],
                                         slab[:, t * P:(t + 1) * P],
                                         wa_sb, start=True, stop=True)
                    stage = ast_pool.tile([P, A_SLAB, 2 * H], BF16, tag="ast")
                    nc.scalar.copy(out=stage[:, :nt, :],
                                   in_=aps[:, :nt, :])
                    nc.sync.dma_start(
                        out=_ap(A_loc, t0 * P * H,
                                [[H, P], [P * H, nt], [1, H]]),
                        in_=stage[:, :nt, H:2 * H])

            # ---------- phase B ----------
            with (
                tc.tile_pool(name="idx", bufs=6) as idx_pool,
                tc.tile_pool(name="gx", bufs=6) as gx_pool,
                tc.tile_pool(name="adb", bufs=4) as adb_pool,
                tc.tile_pool(name="uexp", bufs=4) as u_pool,
                tc.tile_pool(name="uer", bufs=3) as uer_pool,
                tc.tile_pool(name="eq", bufs=4) as eq_pool,
                tc.tile_pool(name="eqt", bufs=3) as eqt_pool,
                tc.tile_pool(name="eqt_ps", bufs=2, space="PSUM") as etp_pool,
                tc.tile_pool(name="ad_ps", bufs=2, space="PSUM") as adp_pool,
                tc.tile_pool(name="rhs", bufs=3) as rhs_pool,
                tc.tile_pool(name="m1", bufs=2, space="PSUM") as m1_pool,
                tc.tile_pool(name="post_ps", bufs=1, space="PSUM") as pps_pool,
                tc.tile_pool(name="post_sb", bufs=3) as psb_pool,
                tc.tile_pool(name="fout", bufs=2) as fout_pool,
            ):
                for b in range(BLOCKS):
                    TPB = tpb_b[b]
                    s_sb = idx_pool.tile([P, TPBMAX * 8], I16, tag="s16")
                    nc.sync.dma_start(
                        out=s_sb[:, :TPB * 8],
                        in_=_ap(src16, boff[b] * 8,
                                [[TOT * 8, P], [1, TPB * 8]]))
                    dl_sb = idx_pool.tile([P, TPBMAX], BF16, tag="dl")
                    nc.sync.dma_start(
                        out=dl_sb[:, :TPB],
                        in_=_ap(dloc, boff[b], [[TOT, P], [1, TPB]]))
                    adb = adb_pool.tile([P, H], BF16, tag="adb")
                    nc.sync.dma_start(out=adb[:],
                                      in_=A_loc[b * P:(b + 1) * P, :])

                    gxb = gx_pool.tile([P, TPBMAX, XW], BF16, tag="gx")
                    woff = 0
                    for w in range(N_WIN):
                        tw = TBW[b][w]
                        if tw == 0:
                            continue
                        rows_w = min(WIN, N_PAD - w * WIN)
                        nc.gpsimd.dma_gather(
                            gxb[:, woff:woff + tw, :],
                            _ap(x_ext, w * WIN * XW,
                                [[XW, rows_w], [1, XW]]),
                            s_sb[:, woff * 8:(woff + tw) * 8],
                            tw * P, tw * P, XW, single_packet=False)
                        woff += tw

                    # eq[p, v, t] = (dl[p, t] == v) — layout B, all packed
                    eqb = eq_pool.tile([P, P, TPBMAX], BF16, tag="eqb")
                    nc.vector.tensor_tensor(
                        out=_ap(eqb.tensor, eqb.offset,
                                [list(eqb.ap[0]), [TPBMAX, P], [1, TPB]]),
                        in0=_ap(dl_sb.tensor, dl_sb.offset,
                                [list(dl_sb.ap[0]), [0, P], [1, TPB]]),
                        in1=_ap(irep.tensor, irep.offset,
                                [list(irep.ap[0]), [TPBMAX, P], [1, TPB]]),
                        op=mybir.AluOpType.is_equal)

                    def eq_t(t):
                        # [p, v] slice for tile t (v columns strided TPBMAX)
                        return _ap(eqb.tensor, eqb.offset + t,
                                   [list(eqb.ap[0]), [TPBMAX, P]])

                    # per-edge a_dst on PE: adst[e, h] = sum_v eqT[v,e] adb[v,h]
                    # transposes batched EQT_BATCH per PSUM bank + one Act copy
                    ad_ps = adp_pool.tile([P, TPBMAX, H], F32, space="PSUM",
                                          tag="adps")
                    n_grp = (TPB + EQT_BATCH - 1) // EQT_BATCH
                    for g in range(n_grp):
                        j0 = g * EQT_BATCH
                        nj = min(EQT_BATCH, TPB - j0)
                        etp = etp_pool.tile([P, EQT_BATCH * P], BF16,
                                            space="PSUM", tag="etp")
                        for j in range(nj):
                            nc.tensor.transpose(
                                etp[:, j * P:(j + 1) * P], eq_t(j0 + j),
                                ident_sb)
                        eqt = eqt_pool.tile([P, EQT_BATCH * P], BF16,
                                            tag="eqt")
                        nc.scalar.copy(out=eqt[:, :nj * P],
                                       in_=etp[:, :nj * P])
                        for j in range(nj):
                            nc.tensor.matmul(ad_ps[:, j0 + j, :],
                                             eqt[:, j * P:(j + 1) * P],
                                             adb[:], start=True, stop=True)

                    # u = exp(leaky_relu(asrc + adst)) for the whole block
                    lg = u_pool.tile([P, TPBMAX, H], F32, tag="lg")
                    lg_s = _ap(lg.tensor, lg.offset,
                               [list(lg.ap[0]), [H, TPB], [1, H]])
                    nc.vector.tensor_add(
                        out=lg_s,
                        in0=_ap(gxb.tensor, gxb.offset + ASRC_COL,
                                [list(gxb.ap[0]), [XW, TPB], [1, H]]),
                        in1=_ap(ad_ps.tensor, ad_ps.offset,
                                [list(ad_ps.ap[0]), [H, TPB], [1, H]]))
                    lr = u_pool.tile([P, TPBMAX, H], F32, tag="lr")
                    lr_s = _ap(lr.tensor, lr.offset,
                               [list(lr.ap[0]), [H, TPB], [1, H]])
                    nc.vector.scalar_tensor_tensor(
                        out=lr_s, in0=lg_s, scalar=NEG_SLOPE, in1=lg_s,
                        op0=mybir.AluOpType.mult, op1=mybir.AluOpType.max)
                    # exp -> uew[..., 0], log-double to UW cols; rhs then
                    # multiplies in D//UW chunks all reusing the same UW
                    # columns of uew (u is constant along c).
                    UW = 16
                    uew = uer_pool.tile([P, TPBMAX, H, UW], BF16, tag="uew")
                    nc.scalar.activation(
                        out=_ap(uew.tensor, uew.offset,
                                [list(uew.ap[0]), [H * UW, TPB], [UW, H]]),
                        in_=lr_s,
                        func=mybir.ActivationFunctionType.Exp)
                    wdt = 1
                    while wdt < UW:
                        nc.vector.tensor_copy(
                            out=_ap(uew.tensor, uew.offset + wdt,
                                    [list(uew.ap[0]), [H * UW, TPB],
                                     [UW, H], [1, wdt]]),
                            in_=_ap(uew.tensor, uew.offset,
                                    [list(uew.ap[0]), [H * UW, TPB],
                                     [UW, H], [1, wdt]]))
                        wdt *= 2
                    # rhs[p, t, h, c] = gx[p, t, c] * u[p, t, h]
                    rhs = rhs_pool.tile([P, TPBMAX, RW], BF16, tag="rhs")
                    for c0 in range(0, D, UW):
                        nc.vector.tensor_mul(
                            out=_ap(rhs.tensor, rhs.offset + c0,
                                    [list(rhs.ap[0]), [RW, TPB],
                                     [D + 1, H], [1, UW]]),
                            in0=_ap(gxb.tensor, gxb.offset + c0,
                                    [list(gxb.ap[0]), [XW, TPB],
                                     [0, H], [1, UW]]),
                            in1=_ap(uew.tensor, uew.offset,
                                    [list(uew.ap[0]), [H * UW, TPB],
                                     [UW, H], [1, UW]]))
                    # ones column: rhs[..., h, D] = u
                    nc.vector.tensor_copy(
                        out=_ap(rhs.tensor, rhs.offset + D,
                                [list(rhs.ap[0]), [RW, TPB], [D + 1, H]]),
                        in_=_ap(uew.tensor, uew.offset,
                                [list(uew.ap[0]), [H * UW, TPB], [UW, H]]))

                    m1_ps = m1_pool.tile([P, RW], F32, space="PSUM", tag="m1")
                    for t in range(TPB):
                        nc.tensor.matmul(
                            m1_ps[:], eq_t(t),
                            _ap(rhs.tensor, rhs.offset + t * RW,
                                [list(rhs.ap[0]), [1, RW]]),
                            start=(t == 0), stop=(t == TPB - 1))

                    # ---- block post ----
                    m1_t = m1_ps.tensor
                    rcp = psb_pool.tile([P, H], F32, tag="rcp")
                    nc.vector.tensor_scalar_add(
                        out=rcp[:],
                        in0=_ap(m1_t, m1_ps.offset + D,
                                [list(m1_ps.ap[0]), [D + 1, H]]),
                        scalar1=1e-16)
                    nc.vector.reciprocal(out=rcp[:], in_=rcp[:])
                    m1n = psb_pool.tile([P, H * D], BF16, tag="m1n")
                    nc.vector.tensor_mul(
                        out=_ap(m1n.tensor, m1n.offset,
                                [list(m1n.ap[0]), [D, H], [1, D]]),
                        in0=_ap(m1_t, m1_ps.offset,
                                [list(m1_ps.ap[0]), [D + 1, H], [1, D]]),
                        in1=_ap(rcp.tensor, rcp.offset,
                                [list(rcp.ap[0]), [1, H], [0, D]]))
                    tp = pps_pool.tile([P, N_CH * P], BF16, space="PSUM",
                                       tag="tp")
                    for ch in range(N_CH):
                        nc.tensor.transpose(
                            tp[:, ch * P:(ch + 1) * P],
                            m1n[:, ch * P:(ch + 1) * P], ident_sb)
                    tps = psb_pool.tile([P, N_CH * P], BF16, tag="tps")
                    nc.scalar.copy(out=tps[:], in_=tp[:])
                    f_ps = pps_pool.tile([P, D], F32, space="PSUM", tag="fps")
                    for ch in range(N_CH):
                        nc.tensor.matmul(f_ps[:], tps[:, ch * P:(ch + 1) * P],
                                         wwl_sb[:, ch * D:(ch + 1) * D],
                                         start=(ch == 0), stop=False)
                    nc.tensor.matmul(f_ps[:], ones_sb[:], blp_sb,
                                     start=False, stop=True)
                    f_sb = fout_pool.tile([P, D], BF16, tag="fsb")
                    nc.scalar.copy(out=f_sb[:], in_=f_ps[:])
                    nc.sync.dma_start(out=out[b * P:(b + 1) * P, :],
                                      in_=f_sb[:])

    nc.compile()
    return nc


def _host_prep(x, edge_index, W, att_src, att_dst, bias, Wl, bl):
    # fused weights (float64 for clean folding)
    Wf = np.asarray(W, np.float64)
    Wlf = np.asarray(Wl, np.float64)
    Was = np.stack([Wf[:, h * D:(h + 1) * D]
                    @ np.asarray(att_src[h], np.float64)
                    for h in range(H)], axis=1)
    Wad = np.stack([Wf[:, h * D:(h + 1) * D]
                    @ np.asarray(att_dst[h], np.float64)
                    for h in range(H)], axis=1)
    Wa = np.concatenate([Was, Wad], axis=1)               # [64, 8]
    WWl_full = np.concatenate(
        [Wf[:, h * D:(h + 1) * D] @ Wlf[h * D:(h + 1) * D, :]
         for h in range(H)], axis=0)                      # [256, 64]
    WWl = np.concatenate([WWl_full[ch * P:(ch + 1) * P, :]
                          for ch in range(N_CH)], axis=1)  # [128, 128]
    blp = (np.asarray(bias, np.float64) @ Wlf
           + np.asarray(bl, np.float64))                  # [64]

    consts = np.zeros((P, 456), NP_BF16)
    consts[:, 0:P] = np.eye(P, dtype=NP_BF16)
    consts[:, P:2 * P] = np.tile(
        np.arange(P, dtype=np.float32).astype(NP_BF16), (P, 1))
    consts[0:D, 256:264] = Wa.astype(NP_BF16)
    consts[:, 264:392] = WWl.astype(NP_BF16)
    consts[0:1, 392:456] = blp.reshape(1, D).astype(NP_BF16)

    # edge tables: sort by dst, then group each block's edges by src window
    src = np.concatenate([np.asarray(edge_index[0]),
                          np.arange(N, dtype=np.int64)]).astype(np.int64)
    dst = np.concatenate([np.asarray(edge_index[1]),
                          np.arange(N, dtype=np.int64)]).astype(np.int64)
    order = np.argsort(dst, kind="stable")
    src = src[order]
    dst = dst[order]
    blk = dst >> 7                                         # 0..NT-1
    win = src >> 15

    # --- balance dst-blocks across cores: group the NT blocks into BLOCKS
    # groups of N_CORES blocks with similar per-window tile counts, so the
    # SPMD per-(slot,window) max padding is minimal. Host unpermutes the
    # output rows afterwards. ---
    cnt_bw = np.zeros((NT, N_WIN), np.int64)
    np.add.at(cnt_bw, (blk, win), 1)
    tiles_b = (cnt_bw + P - 1) // P                        # [NT, N_WIN]
    order_blocks = np.lexsort((tiles_b[:, 3], tiles_b[:, 2],
                               tiles_b[:, 1], tiles_b[:, 0]))[::-1]
    assign = order_blocks.reshape(BLOCKS, N_CORES)         # [slot, core]
    core_of = np.zeros(NT, np.int64)
    slot_of = np.zeros(NT, np.int64)
    for s in range(BLOCKS):
        for c in range(N_CORES):
            core_of[assign[s, c]] = c
            slot_of[assign[s, c]] = s
    tiles_bw = tiles_b[assign].max(axis=1)                 # [BLOCKS, N_WIN]
    TBW = tuple(tuple(int(x) for x in row) for row in tiles_bw)

    core = core_of[blk]
    b_loc = slot_of[blk]
    key = (core * BLOCKS + b_loc) * N_WIN + win
    order2 = np.argsort(key, kind="stable")
    src, dst, key, win = src[order2], dst[order2], key[order2], win[order2]
    core, b_loc = core[order2], b_loc[order2]
    run_cnt = np.bincount(key, minlength=N_CORES * BLOCKS * N_WIN)
    woff = np.zeros((BLOCKS, N_WIN), np.int64)
    woff[:, 1:] = np.cumsum(tiles_bw[:, :-1], axis=1)
    tpb_b = tiles_bw.sum(axis=1)
    boff = np.zeros(BLOCKS + 1, np.int64)
    np.cumsum(tpb_b, out=boff[1:])
    TOT = int(boff[-1])
    run_starts = np.zeros(len(run_cnt) + 1, np.int64)
    np.cumsum(run_cnt, out=run_starts[1:])

    jr = np.arange(len(dst), dtype=np.int64) - run_starts[key]
    p = jr % P
    tile_g = boff[b_loc] + woff[b_loc, win] + jr // P      # per-core tile id

    sv = np.zeros((N_CORES, TOT * P), np.int64)
    sv[core, tile_g * P + p] = src - win * WIN
    dl8 = np.full((N_CORES, P, TOT), 255.0, np.float32)
    dl8[core, p, tile_g] = (dst & 127).astype(np.float32)
    dl8 = dl8.astype(NP_BF16)

    def wrap16(v):     # [C, TOT*128] -> [C, 128, TOT*8]
        a = v.reshape(N_CORES, TOT, 8, 16).astype(np.int16)
        a = a.transpose(0, 3, 1, 2).reshape(N_CORES, 16, TOT * 8)
        return np.tile(a, (1, 8, 1))

    src16 = wrap16(sv)

    x_np = np.asarray(x, np.float32)
    x_ext = np.zeros((N_PAD, XW), NP_BF16)
    x_ext[:N, :D] = x_np.astype(NP_BF16)
    x_ext[:, ONE_COL] = np.float32(1.0).astype(NP_BF16)
    xT = np.zeros((D, N_PAD), NP_BF16)
    xT[:, :N] = x_np.T.astype(NP_BF16)

    shared = {"x_ext": x_ext, "xT": xT, "consts": consts}
    percore = []
    for c in range(N_CORES):
        # own-node columns of xT in slot order (phase A2 -> A_loc rows)
        node_cols = (assign[:, c][:, None] * P
                     + np.arange(P)[None, :]).reshape(-1)  # [NLOC]
        percore.append({
            "src16": src16[c], "dloc": dl8[c],
            "xTd": np.ascontiguousarray(xT[:, node_cols]),
        })
    return shared, percore, TBW, assign


_PROG_CACHE = {}
LAST_EXEC_NS = None
CHAIN_K = int(os.environ.get("BASS_GAT_CHAIN_K", "64"))


def _run_pjrt(nc, in_maps, n_cores, bench=True):
    """Execute via PJRT (axon). For benchmarking, chain CHAIN_K kernel
    executions inside one launch (each call consumes the previous call's
    outputs as its output-buffer operands, forcing serial execution on
    device); HW exec time per kernel = (T_chain - T_single)/(K - 1)."""
    import time
    import jax
    from jax.experimental.shard_map import shard_map
    from jax.sharding import Mesh, PartitionSpec, NamedSharding
    from concourse import bass2jax, mybir as mb

    bass2jax.install_neuronx_cc_hook()
    partition_name = (nc.partition_id_tensor.name
                      if nc.partition_id_tensor else None)

    in_names, out_names, out_avals, zero_outs = [], [], [], []
    for alloc in nc.m.functions[0].allocations:
        if not isinstance(alloc, mb.MemoryLocationSet):
            continue
        name = alloc.memorylocations[0].name
        if alloc.kind == "ExternalInput":
            if name != partition_name:
                in_names.append(name)
        elif alloc.kind == "ExternalOutput":
            shape = tuple(alloc.tensor_shape)
            dtype = mb.dt.np(alloc.dtype)
            out_names.append(name)
            out_avals.append(jax.core.ShapedArray(shape, dtype))
            zero_outs.append(np.zeros(shape, dtype))
    n_params = len(in_names)
    all_in_names = in_names + out_names + ([partition_name]
                                           if partition_name else [])

    def _make_body(k):
        def _body(*args):
            ins = list(args[:n_params])
            outs = list(args[n_params:])
            for _ in range(k):
                operands = ins + outs
                if partition_name is not None:
                    operands.append(bass2jax.partition_id_tensor())
                outs = list(bass2jax._bass_exec_p.bind(
                    *operands,
                    out_avals=tuple(out_avals),
                    in_names=tuple(all_in_names),
                    out_names=tuple(out_names),
                    lowering_input_output_aliases=(),
                    sim_require_finite=True,
                    sim_require_nnan=True,
                    nc=nc,
                ))
            return tuple(outs)
        return _body

    devices = jax.devices()[:n_cores]
    mesh = Mesh(np.asarray(devices), ("core",))
    n_outs = len(out_names)
    specs = (PartitionSpec("core"),) * (n_params + n_outs)
    o_specs = (PartitionSpec("core"),) * n_outs
    run1 = jax.jit(
        shard_map(_make_body(1), mesh=mesh, in_specs=specs,
                  out_specs=o_specs, check_rep=False),
        keep_unused=True)
    concat_in = [
        np.concatenate([np.asarray(in_maps[c][nm]) for c in range(n_cores)],
                       axis=0)
        for nm in in_names
    ]
    concat_zeros = [np.zeros((n_cores * z.shape[0], *z.shape[1:]), z.dtype)
                    for z in zero_outs]
    shard = NamedSharding(mesh, PartitionSpec("core"))
    dev_args = [jax.device_put(a, shard)
                for a in (*concat_in, *concat_zeros)]
    out_arrs = run1(*dev_args)
    jax.block_until_ready(out_arrs)

    best_ns = None
    if bench:
        # pipelined-launch throughput: back-to-back async launches overlap
        # the ~85ms axon round-trip; the marginal cost per launch is the
        # device execution (+ per-launch dispatch overhead). Report the
        # slope between a short and a long pipeline, which cancels the
        # fixed round-trip latency.
        from concurrent.futures import ThreadPoolExecutor
        M1, M2 = 8, CHAIN_K

        def _pipe(m):
            t0 = time.perf_counter_ns()
            rs = [run1(*dev_args) for _ in range(m)]
            jax.block_until_ready(rs)
            return time.perf_counter_ns() - t0

        def _pipe_t(m, nthreads=4):
            t0 = time.perf_counter_ns()
            with ThreadPoolExecutor(nthreads) as ex:
                rs = list(ex.map(lambda _: run1(*dev_args), range(m)))
            jax.block_until_ready(rs)
            return time.perf_counter_ns() - t0

        slopes, tslopes = [], []
        try:
            # sustained warmup (~2s) so device clocks settle
            for _ in range(3):
                _pipe(CHAIN_K)
            for _ in range(6):
                ta = _pipe(M1)
                tb = _pipe(M2)
                slopes.append((tb - ta) / (M2 - M1))
            for _ in range(2):
                ta = _pipe_t(M1)
                tb = _pipe_t(M2)
                tslopes.append((tb - ta) / (M2 - M1))
        except Exception as e:
            print(f"[bench] pipeline bench failed: {e}", flush=True)
        print(f"[bench] pipelined slope per launch: "
              f"{[int(s) for s in slopes]} ns; threaded: "
              f"{[int(s) for s in tslopes]} ns", flush=True)
        if slopes + tslopes:
            best_ns = max(1, int(round(min(slopes + tslopes))))
        else:
            # fallback: single blocking launch (includes the full RTT)
            t0 = time.perf_counter_ns()
            jax.block_until_ready(run1(*dev_args))
            best_ns = time.perf_counter_ns() - t0

    results = [
        {nm: np.asarray(out_arrs[i]).reshape(n_cores, *out_avals[i].shape)[c]
         for i, nm in enumerate(out_names)}
        for c in range(n_cores)
    ]
    return results, best_ns


def kernel(x, edge_index, W, att_src, att_dst, bias, Wl, bl):
    global LAST_EXEC_NS
    shared, percore, TW, assign = _host_prep(
        x, edge_index, W, att_src, att_dst, bias, Wl, bl)

    if TW not in _PROG_CACHE:
        _PROG_CACHE[TW] = build_program(TW)
    nc = _PROG_CACHE[TW]

    in_maps = [dict(shared, **percore[c]) for c in range(N_CORES)]

    if os.environ.get("BASS_GAT_SIM"):
        from concourse.bass_interp import CoreSim
        outs = []
        for c in range(int(os.environ.get("BASS_GAT_SIM_CORES", N_CORES))):
            sim = CoreSim(nc)
            for k, v in in_maps[c].items():
                sim.tensor(k)[:] = v
            sim.simulate()
            outs.append(np.array(sim.tensor("out")))
        while len(outs) < N_CORES:
            outs.append(np.zeros((NLOC, D), np.float32))
    else:
        bench = os.environ.get("BASS_GAT_BENCH", "1") != "0"
        results, best_ns = _run_pjrt(nc, in_maps, N_CORES, bench=bench)
        outs = [r["out"] for r in results]
        LAST_EXEC_NS = best_ns
    # unpermute: core c's slot s holds dst-block assign[s, c]
    full = np.zeros((NT, P, D), np.float32)
    for c in range(N_CORES):
        full[assign[:, c]] = np.asarray(outs[c]).reshape(BLOCKS, P, D)
    full = full.reshape(NT * P, D)[:N]
    return np.ascontiguousarray(full.astype(np.float32))


# revision 31
# speedup vs baseline: 1.0781x; 1.0781x over previous
"""GAT multi-head block on 8 Trainium2 NeuronCores — v4.

Edge-parallel, dst-sharded (as v3) with DVE/Act-focused optimizations:
  * eq one-hot built in [p, v, t] layout (all APs packed innermost) ->
    DVE 2x mode: 2994ns -> 1527ns per block.
  * rhs = u (x) [x|1]: ue log-expanded to ue_rep[p,t,h,65] via packed
    TensorCopies (4x mode), then a fully-packed mult (2x mode):
    6019ns -> ~1900+3040ns per block.
  * per-tile eqT PSUM->SBUF copies batched 8 tiles per Activation copy
    (amortizes the 287ns PSUM access penalty): 292ns/tile -> ~143ns/tile.
  * phase A matmuls accumulate 64 tiles into one PSUM bank, single
    staged copy per slab instead of per-tile copies.
  * post-block transposes copied in one batched Act copy.
Host side: dst-blocks are load-balanced across cores (lexsorted by
per-window tile vectors, grouped 8 similar blocks per SPMD slot) to
minimize the shared-program padding; outputs are unpermuted on host.
Bench: back-to-back async launches overlap the ~85ms axon round-trip;
HW exec time = marginal wall time per launch (slope between an 8-deep
and a 64-deep pipeline), i.e. device execution + per-launch overhead.
"""

import os
import sys
import numpy as np

for _p in ("/opt/trn_rl_repo",):
    if _p not in sys.path:
        sys.path.insert(0, _p)

import concourse.bass as bass
import concourse.bacc as bacc
import concourse.mybir as mybir
import concourse.tile as tile

F32 = mybir.dt.float32
BF16 = mybir.dt.bfloat16
I16 = mybir.dt.int16
NP_BF16 = np.dtype(mybir.dt.np(BF16))

NEG_SLOPE = 0.2
P = 128
N_CORES = 8
D = 64
H = 4
N = 100000
NBLK_TOTAL = (N + P - 1) // P                     # 782
BLOCKS = (NBLK_TOTAL + N_CORES - 1) // N_CORES    # 98
N_PAD = BLOCKS * N_CORES * P                      # 100352
NT = N_PAD // P                                   # 784
NLOC = BLOCKS * P                                 # 12544
WIN = 32768
N_WIN = (N_PAD + WIN - 1) // WIN                  # 4
XW = 128                                          # 256B gather rows
ONE_COL = D                                       # 64
ASRC_COL = 66
RW = H * (D + 1)                                  # 260
N_CH = H * D // P                                 # 2
EQT_BATCH = 8                                     # tiles per Act copy


def _ap(t, offset_elems, dims):
    return bass.AP(t, offset_elems, [list(d) for d in dims])


def build_program(TBW):
    # TBW: per-block tuple of per-window tile counts (max over cores)
    tpb_b = [sum(row) for row in TBW]
    TPBMAX = max(tpb_b)
    boff = [0]
    for t in tpb_b:
        boff.append(boff[-1] + t)
    TOT = boff[-1]

    nc = bacc.Bacc("TRN2", target_bir_lowering=False, debug=False,
                   num_devices=N_CORES)

    # x_ext split per src-window so window-w gathers only wait on the
    # phase-A a_src writes for that window (DRAM deps are tensor-granular)
    WROWS = [min(WIN, N_PAD - w * WIN) for w in range(N_WIN)]
    x_ext_w = [nc.declare_dram_parameter(f"x_ext{w}", [WROWS[w], XW], BF16,
                                         isOutput=False)
               for w in range(N_WIN)]
    xT = nc.declare_dram_parameter("xT", [D, N_PAD], BF16, isOutput=False)
    xTd = nc.declare_dram_parameter("xTd", [D, NLOC], BF16, isOutput=False)
    # consts [128, 456]: ident | iota | Wa(rows 0:64) | WWl | blp(row 0)
    C_IDENT, C_IOTA, C_WA, C_WWL, C_BLP = 0, 128, 256, 264, 392
    consts = nc.declare_dram_parameter("consts", [P, 456], BF16,
                                       isOutput=False)
    src16 = nc.declare_dram_parameter("src16", [P, TOT * 8], I16,
                                      isOutput=False)
    dloc = nc.declare_dram_parameter("dloc", [P, TOT], BF16,
                                     isOutput=False)
    out = nc.declare_dram_parameter("out", [NLOC, D], BF16, isOutput=True)
    A_loc = nc.dram_tensor("A_loc", [NLOC, H], BF16)

    A_SLAB = 64
    n_slabs = (NT + A_SLAB - 1) // A_SLAB

    with tile.TileContext(nc) as tc:
        with tc.tile_pool(name="const", bufs=1) as cpool:
            c_sb = cpool.tile([P, 456], BF16, tag="consts")
            nc.sync.dma_start(out=c_sb[:], in_=consts[:])
            ones_sb = cpool.tile([1, P], BF16, tag="ones")
            nc.vector.memset(ones_sb[:], 1.0)
            # iota_rep[p, v, t] = v  (bf16 exact for 0..127)
            irep = cpool.tile([P, P, TPBMAX], BF16, tag="irep")
            nc.gpsimd.iota(irep[:], pattern=[[1, P], [0, TPBMAX]], base=0,
                           channel_multiplier=0,
                           allow_small_or_imprecise_dtypes=True)
            ident_sb = c_sb[:, C_IDENT:C_IDENT + P]
            wa_sb = c_sb[0:D, C_WA:C_WA + 2 * H]
            wwl_sb = c_sb[:, C_WWL:C_WWL + N_CH * D]
            blp_sb = c_sb[0:1, C_BLP:C_BLP + D]

            # single pool scope for both phases: a pool-scope exit drains all
            # engines, which would serialize phase B behind phase A and waste
            # the per-window dependency split of x_ext.
            import contextlib
            with contextlib.ExitStack() as stk:
                xt_pool = stk.enter_context(tc.tile_pool(name="a_xt", bufs=2))
                aps_pool = stk.enter_context(
                    tc.tile_pool(name="a_ps", bufs=1, space="PSUM"))
                ast_pool = stk.enter_context(tc.tile_pool(name="a_st", bufs=2))
                idx_pool = stk.enter_context(tc.tile_pool(name="idx", bufs=6))
                gx_pool = stk.enter_context(tc.tile_pool(name="gx", bufs=6))
                adb_pool = stk.enter_context(tc.tile_pool(name="adb", bufs=4))
                u_pool = stk.enter_context(tc.tile_pool(name="uexp", bufs=4))
                uer_pool = stk.enter_context(tc.tile_pool(name="uer", bufs=3))
                eq_pool = stk.enter_context(tc.tile_pool(name="eq", bufs=4))
                eqt_pool = stk.enter_context(tc.tile_pool(name="eqt", bufs=3))
                etp_pool = stk.enter_context(
                    tc.tile_pool(name="eqt_ps", bufs=2, space="PSUM"))
                adp_pool = stk.enter_context(
                    tc.tile_pool(name="ad_ps", bufs=2, space="PSUM"))
                rhs_pool = stk.enter_context(tc.tile_pool(name="rhs", bufs=3))
                m1_pool = stk.enter_context(
                    tc.tile_pool(name="m1", bufs=2, space="PSUM"))
                pps_pool = stk.enter_context(
                    tc.tile_pool(name="post_ps", bufs=1, space="PSUM"))
                psb_pool = stk.enter_context(
                    tc.tile_pool(name="post_sb", bufs=3))
                fout_pool = stk.enter_context(tc.tile_pool(name="fout", bufs=2))
                # ------ phase A: [a_src | a_dst] = x @ Wa ------
                # A1: a_src for all nodes -> x_ext cols 66:70
                # A_SLAB=64 tiles = 8192 rows; windows are 4 slabs each, so
                # each slab's a_src write targets exactly one x_ext{w}.
                for s in range(n_slabs):
                    t0 = s * A_SLAB
                    nt = min(A_SLAB, NT - t0)
                    w = (t0 * P) // WIN
                    row0 = t0 * P - w * WIN
                    slab = xt_pool.tile([D, A_SLAB * P], BF16, tag="slab")
                    nc.sync.dma_start(out=slab[:, : nt * P],
                                      in_=xT[:, t0 * P:(t0 + nt) * P])
                    aps = aps_pool.tile([P, A_SLAB, 2 * H], F32, space="PSUM",
                                        tag="aps")
                    for t in range(nt):
                        nc.tensor.matmul(aps[:, t, :],
                                         slab[:, t * P:(t + 1) * P],
                                         wa_sb, start=True, stop=True)
                    stage = ast_pool.tile([P, A_SLAB, 2 * H], BF16, tag="ast")
                    nc.scalar.copy(out=stage[:, :nt, :],
                                   in_=aps[:, :nt, :])
                    nc.scalar.dma_start(
                        out=_ap(x_ext_w[w], row0 * XW + ASRC_COL,
                                [[XW, P], [P * XW, nt], [1, H]]),
                        in_=stage[:, :nt, 0:H])
                # A2: a_dst for this core's own nodes -> A_loc cols 0:4
                NTd = NLOC // P
                n_slabs_d = (NTd + A_SLAB - 1) // A_SLAB
                for s in range(n_slabs_d):
                    t0 = s * A_SLAB
                    nt = min(A_SLAB, NTd - t0)
                    slab = xt_pool.tile([D, A_SLAB * P], BF16, tag="slab")
                    nc.sync.dma_start(out=slab[:, : nt * P],
                                      in_=xTd[:, t0 * P:(t0 + nt) * P])
                    aps = aps_pool.tile([P, A_SLAB, 2 * H], F32, space="PSUM",
                                        tag="aps")
                    for t in range(nt):
                        nc.tensor.matmul(aps[:, t, :],
                                         slab[:, t * P:(t + 1) * P],
                                         wa_sb, start=True, stop=True)
                    stage = ast_pool.tile([P, A_SLAB, 2 * H], BF16, tag="ast")
                    nc.scalar.copy(out=stage[:, :nt, :],
                                   in_=aps[:, :nt, :])
                    nc.scalar.dma_start(
                        out=_ap(A_loc, t0 * P * H,
                                [[H, P], [P * H, nt], [1, H]]),
                        in_=stage[:, :nt, H:2 * H])

                # ------ phase B ------
                for b in range(BLOCKS):
                    TPB = tpb_b[b]
                    s_sb = idx_pool.tile([P, TPBMAX * 8], I16, tag="s16")
                    nc.sync.dma_start(
                        out=s_sb[:, :TPB * 8],
                        in_=_ap(src16, boff[b] * 8,
                                [[TOT * 8, P], [1, TPB * 8]]))
                    dl_sb = idx_pool.tile([P, TPBMAX], BF16, tag="dl")
                    nc.sync.dma_start(
                        out=dl_sb[:, :TPB],
                        in_=_ap(dloc, boff[b], [[TOT, P], [1, TPB]]))
                    adb = adb_pool.tile([P, H], BF16, tag="adb")
                    nc.sync.dma_start(out=adb[:],
                                      in_=A_loc[b * P:(b + 1) * P, :])

                    gxb = gx_pool.tile([P, TPBMAX, XW], BF16, tag="gx")
                    woff = 0
                    for w in range(N_WIN):
                        tw = TBW[b][w]
                        if tw == 0:
                            continue
                        nc.gpsimd.dma_gather(
                            gxb[:, woff:woff + tw, :],
                            _ap(x_ext_w[w], 0,
                                [[XW, WROWS[w]], [1, XW]]),
                            s_sb[:, woff * 8:(woff + tw) * 8],
                            tw * P, tw * P, XW, single_packet=False)
                        woff += tw

                    # eq[p, v, t] = (dl[p, t] == v) — layout B, all packed
                    eqb = eq_pool.tile([P, P, TPBMAX], BF16, tag="eqb")
                    nc.vector.tensor_tensor(
                        out=_ap(eqb.tensor, eqb.offset,
                                [list(eqb.ap[0]), [TPBMAX, P], [1, TPB]]),
                        in0=_ap(dl_sb.tensor, dl_sb.offset,
                                [list(dl_sb.ap[0]), [0, P], [1, TPB]]),
                        in1=_ap(irep.tensor, irep.offset,
                                [list(irep.ap[0]), [TPBMAX, P], [1, TPB]]),
                        op=mybir.AluOpType.is_equal)

                    def eq_t(t):
                        # [p, v] slice for tile t (v columns strided TPBMAX)
                        return _ap(eqb.tensor, eqb.offset + t,
                                   [list(eqb.ap[0]), [TPBMAX, P]])

                    # per-edge a_dst on PE: adst[e, h] = sum_v eqT[v,e] adb[v,h]
                    # transposes batched EQT_BATCH per PSUM bank + one Act copy
                    ad_ps = adp_pool.tile([P, TPBMAX, H], F32, space="PSUM",
                                          tag="adps")
                    n_grp = (TPB + EQT_BATCH - 1) // EQT_BATCH
                    for g in range(n_grp):
                        j0 = g * EQT_BATCH
                        nj = min(EQT_BATCH, TPB - j0)
                        etp = etp_pool.tile([P, EQT_BATCH * P], BF16,
                                            space="PSUM", tag="etp")
                        for j in range(nj):
                            nc.tensor.transpose(
                                etp[:, j * P:(j + 1) * P], eq_t(j0 + j),
                                ident_sb)
                        eqt = eqt_pool.tile([P, EQT_BATCH * P], BF16,
                                            tag="eqt")
                        nc.scalar.copy(out=eqt[:, :nj * P],
                                       in_=etp[:, :nj * P])
                        for j in range(nj):
                            nc.tensor.matmul(ad_ps[:, j0 + j, :],
                                             eqt[:, j * P:(j + 1) * P],
                                             adb[:], start=True, stop=True)

                    # u = exp(leaky_relu(asrc + adst)) for the whole block
                    lg = u_pool.tile([P, TPBMAX, H], F32, tag="lg")
                    lg_s = _ap(lg.tensor, lg.offset,
                               [list(lg.ap[0]), [H, TPB], [1, H]])
                    nc.vector.tensor_add(
                        out=lg_s,
                        in0=_ap(gxb.tensor, gxb.offset + ASRC_COL,
                                [list(gxb.ap[0]), [XW, TPB], [1, H]]),
                        in1=_ap(ad_ps.tensor, ad_ps.offset,
                                [list(ad_ps.ap[0]), [H, TPB], [1, H]]))
                    lr = u_pool.tile([P, TPBMAX, H], F32, tag="lr")
                    lr_s = _ap(lr.tensor, lr.offset,
                               [list(lr.ap[0]), [H, TPB], [1, H]])
                    nc.vector.scalar_tensor_tensor(
                        out=lr_s, in0=lg_s, scalar=NEG_SLOPE, in1=lg_s,
                        op0=mybir.AluOpType.mult, op1=mybir.AluOpType.max)
                    # exp -> uew[..., 0], log-double to UW cols; rhs then
                    # multiplies in D//UW chunks all reusing the same UW
                    # columns of uew (u is constant along c).
                    UW = 16
                    uew = uer_pool.tile([P, TPBMAX, H, UW], BF16, tag="uew")
                    nc.scalar.activation(
                        out=_ap(uew.tensor, uew.offset,
                                [list(uew.ap[0]), [H * UW, TPB], [UW, H]]),
                        in_=lr_s,
                        func=mybir.ActivationFunctionType.Exp)
                    wdt = 1
                    while wdt < UW:
                        nc.vector.tensor_copy(
                            out=_ap(uew.tensor, uew.offset + wdt,
                                    [list(uew.ap[0]), [H * UW, TPB],
                                     [UW, H], [1, wdt]]),
                            in_=_ap(uew.tensor, uew.offset,
                                    [list(uew.ap[0]), [H * UW, TPB],
                                     [UW, H], [1, wdt]]))
                        wdt *= 2
                    # rhs[p, t, h, c] = gx[p, t, c] * u[p, t, h]
                    rhs = rhs_pool.tile([P, TPBMAX, RW], BF16, tag="rhs")
                    for c0 in range(0, D, UW):
                        nc.vector.tensor_mul(
                            out=_ap(rhs.tensor, rhs.offset + c0,
                                    [list(rhs.ap[0]), [RW, TPB],
                                     [D + 1, H], [1, UW]]),
                            in0=_ap(gxb.tensor, gxb.offset + c0,
                                    [list(gxb.ap[0]), [XW, TPB],
                                     [0, H], [1, UW]]),
                            in1=_ap(uew.tensor, uew.offset,
                                    [list(uew.ap[0]), [H * UW, TPB],
                                     [UW, H], [1, UW]]))
                    # ones column: rhs[..., h, D] = u
                    nc.vector.tensor_copy(
                        out=_ap(rhs.tensor, rhs.offset + D,
                                [list(rhs.ap[0]), [RW, TPB], [D + 1, H]]),
                        in_=_ap(uew.tensor, uew.offset,
                                [list(uew.ap[0]), [H * UW, TPB], [UW, H]]))

                    m1_ps = m1_pool.tile([P, RW], F32, space="PSUM", tag="m1")
                    for t in range(TPB):
                        nc.tensor.matmul(
                            m1_ps[:], eq_t(t),
                            _ap(rhs.tensor, rhs.offset + t * RW,
                                [list(rhs.ap[0]), [1, RW]]),
                            start=(t == 0), stop=(t == TPB - 1))

                    # ---- block post ----
                    m1_t = m1_ps.tensor
                    rcp = psb_pool.tile([P, H], F32, tag="rcp")
                    nc.vector.tensor_scalar_add(
                        out=rcp[:],
                        in0=_ap(m1_t, m1_ps.offset + D,
                                [list(m1_ps.ap[0]), [D + 1, H]]),
                        scalar1=1e-16)
                    nc.vector.reciprocal(out=rcp[:], in_=rcp[:])
                    m1n = psb_pool.tile([P, H * D], BF16, tag="m1n")
                    nc.vector.tensor_mul(
                        out=_ap(m1n.tensor, m1n.offset,
                                [list(m1n.ap[0]), [D, H], [1, D]]),
                        in0=_ap(m1_t, m1_ps.offset,
                                [list(m1_ps.ap[0]), [D + 1, H], [1, D]]),
                        in1=_ap(rcp.tensor, rcp.offset,
                                [list(rcp.ap[0]), [1, H], [0, D]]))
                    tp = pps_pool.tile([P, N_CH * P], BF16, space="PSUM",
                                       tag="tp")
                    for ch in range(N_CH):
                        nc.tensor.transpose(
                            tp[:, ch * P:(ch + 1) * P],
                            m1n[:, ch * P:(ch + 1) * P], ident_sb)
                    tps = psb_pool.tile([P, N_CH * P], BF16, tag="tps")
                    nc.scalar.copy(out=tps[:], in_=tp[:])
                    # reuse the phase-A aps PSUM bank (disjoint lifetime)
                    f_tile = aps_pool.tile([P, A_SLAB, 2 * H], F32,
                                           space="PSUM", tag="aps")
                    f_ps = _ap(f_tile.tensor, f_tile.offset,
                               [list(f_tile.ap[0]), [1, D]])
                    for ch in range(N_CH):
                        nc.tensor.matmul(f_ps, tps[:, ch * P:(ch + 1) * P],
                                         wwl_sb[:, ch * D:(ch + 1) * D],
                                         start=(ch == 0), stop=False)
                    nc.tensor.matmul(f_ps, ones_sb[:], blp_sb,
                                     start=False, stop=True)
                    f_sb = fout_pool.tile([P, D], BF16, tag="fsb")
                    nc.scalar.copy(out=f_sb[:], in_=f_ps)
                    nc.sync.dma_start(out=out[b * P:(b + 1) * P, :],
                                      in_=f_sb[:])

    nc.compile()
    return nc


def _host_prep(x, edge_index, W, att_src, att_dst, bias, Wl, bl):
    # fused weights (float64 for clean folding)
    Wf = np.asarray(W, np.float64)
    Wlf = np.asarray(Wl, np.float64)
    Was = np.stack([Wf[:, h * D:(h + 1) * D]
                    @ np.asarray(att_src[h], np.float64)
                    for h in range(H)], axis=1)
    Wad = np.stack([Wf[:, h * D:(h + 1) * D]
                    @ np.asarray(att_dst[h], np.float64)
                    for h in range(H)], axis=1)
    Wa = np.concatenate([Was, Wad], axis=1)               # [64, 8]
    WWl_full = np.concatenate(
        [Wf[:, h * D:(h + 1) * D] @ Wlf[h * D:(h + 1) * D, :]
         for h in range(H)], axis=0)                      # [256, 64]
    WWl = np.concatenate([WWl_full[ch * P:(ch + 1) * P, :]
                          for ch in range(N_CH)], axis=1)  # [128, 128]
    blp = (np.asarray(bias, np.float64) @ Wlf
           + np.asarray(bl, np.float64))                  # [64]

    consts = np.zeros((P, 456), NP_BF16)
    consts[:, 0:P] = np.eye(P, dtype=NP_BF16)
    consts[:, P:2 * P] = np.tile(
        np.arange(P, dtype=np.float32).astype(NP_BF16), (P, 1))
    consts[0:D, 256:264] = Wa.astype(NP_BF16)
    consts[:, 264:392] = WWl.astype(NP_BF16)
    consts[0:1, 392:456] = blp.reshape(1, D).astype(NP_BF16)

    # edge tables: sort by dst, then group each block's edges by src window
    src = np.concatenate([np.asarray(edge_index[0]),
                          np.arange(N, dtype=np.int64)]).astype(np.int64)
    dst = np.concatenate([np.asarray(edge_index[1]),
                          np.arange(N, dtype=np.int64)]).astype(np.int64)
    order = np.argsort(dst, kind="stable")
    src = src[order]
    dst = dst[order]
    blk = dst >> 7                                         # 0..NT-1
    win = src >> 15

    # --- balance dst-blocks across cores: group the NT blocks into BLOCKS
    # groups of N_CORES blocks with similar per-window tile counts, so the
    # SPMD per-(slot,window) max padding is minimal. Host unpermutes the
    # output rows afterwards. ---
    cnt_bw = np.zeros((NT, N_WIN), np.int64)
    np.add.at(cnt_bw, (blk, win), 1)
    tiles_b = (cnt_bw + P - 1) // P                        # [NT, N_WIN]
    order_blocks = np.lexsort((tiles_b[:, 3], tiles_b[:, 2],
                               tiles_b[:, 1], tiles_b[:, 0]))[::-1]
    assign = order_blocks.reshape(BLOCKS, N_CORES)         # [slot, core]
    core_of = np.zeros(NT, np.int64)
    slot_of = np.zeros(NT, np.int64)
    for s in range(BLOCKS):
        for c in range(N_CORES):
            core_of[assign[s, c]] = c
            slot_of[assign[s, c]] = s
    tiles_bw = tiles_b[assign].max(axis=1)                 # [BLOCKS, N_WIN]
    TBW = tuple(tuple(int(x) for x in row) for row in tiles_bw)

    core = core_of[blk]
    b_loc = slot_of[blk]
    key = (core * BLOCKS + b_loc) * N_WIN + win
    order2 = np.argsort(key, kind="stable")
    src, dst, key, win = src[order2], dst[order2], key[order2], win[order2]
    core, b_loc = core[order2], b_loc[order2]
    run_cnt = np.bincount(key, minlength=N_CORES * BLOCKS * N_WIN)
    woff = np.zeros((BLOCKS, N_WIN), np.int64)
    woff[:, 1:] = np.cumsum(tiles_bw[:, :-1], axis=1)
    tpb_b = tiles_bw.sum(axis=1)
    boff = np.zeros(BLOCKS + 1, np.int64)
    np.cumsum(tpb_b, out=boff[1:])
    TOT = int(boff[-1])
    run_starts = np.zeros(len(run_cnt) + 1, np.int64)
    np.cumsum(run_cnt, out=run_starts[1:])

    jr = np.arange(len(dst), dtype=np.int64) - run_starts[key]
    p = jr % P
    tile_g = boff[b_loc] + woff[b_loc, win] + jr // P      # per-core tile id

    sv = np.zeros((N_CORES, TOT * P), np.int64)
    sv[core, tile_g * P + p] = src - win * WIN
    dl8 = np.full((N_CORES, P, TOT), 255.0, np.float32)
    dl8[core, p, tile_g] = (dst & 127).astype(np.float32)
    dl8 = dl8.astype(NP_BF16)

    def wrap16(v):     # [C, TOT*128] -> [C, 128, TOT*8]
        a = v.reshape(N_CORES, TOT, 8, 16).astype(np.int16)
        a = a.transpose(0, 3, 1, 2).reshape(N_CORES, 16, TOT * 8)
        return np.tile(a, (1, 8, 1))

    src16 = wrap16(sv)

    x_np = np.asarray(x, np.float32)
    x_ext = np.zeros((N_PAD, XW), NP_BF16)
    x_ext[:N, :D] = x_np.astype(NP_BF16)
    x_ext[:, ONE_COL] = np.float32(1.0).astype(NP_BF16)
    xT = np.zeros((D, N_PAD), NP_BF16)
    xT[:, :N] = x_np.T.astype(NP_BF16)

    shared = {"xT": xT, "consts": consts}
    for w in range(N_WIN):
        shared[f"x_ext{w}"] = np.ascontiguousarray(
            x_ext[w * WIN:min((w + 1) * WIN, N_PAD)])
    percore = []
    for c in range(N_CORES):
        # own-node columns of xT in slot order (phase A2 -> A_loc rows)
        node_cols = (assign[:, c][:, None] * P
                     + np.arange(P)[None, :]).reshape(-1)  # [NLOC]
        percore.append({
            "src16": src16[c], "dloc": dl8[c],
            "xTd": np.ascontiguousarray(xT[:, node_cols]),
        })
    return shared, percore, TBW, assign


_PROG_CACHE = {}
LAST_EXEC_NS = None
CHAIN_K = int(os.environ.get("BASS_GAT_CHAIN_K", "64"))


def _run_pjrt(nc, in_maps, n_cores, bench=True):
    """Execute via PJRT (axon). For benchmarking, chain CHAIN_K kernel
    executions inside one launch (each call consumes the previous call's
    outputs as its output-buffer operands, forcing serial execution on
    device); HW exec time per kernel = (T_chain - T_single)/(K - 1)."""
    import time
    import jax
    from jax.experimental.shard_map import shard_map
    from jax.sharding import Mesh, PartitionSpec, NamedSharding
    from concourse import bass2jax, mybir as mb

    bass2jax.install_neuronx_cc_hook()
    partition_name = (nc.partition_id_tensor.name
                      if nc.partition_id_tensor else None)

    in_names, out_names, out_avals, zero_outs = [], [], [], []
    for alloc in nc.m.functions[0].allocations:
        if not isinstance(alloc, mb.MemoryLocationSet):
            continue
        name = alloc.memorylocations[0].name
        if alloc.kind == "ExternalInput":
            if name != partition_name:
                in_names.append(name)
        elif alloc.kind == "ExternalOutput":
            shape = tuple(alloc.tensor_shape)
            dtype = mb.dt.np(alloc.dtype)
            out_names.append(name)
            out_avals.append(jax.core.ShapedArray(shape, dtype))
            zero_outs.append(np.zeros(shape, dtype))
    n_params = len(in_names)
    all_in_names = in_names + out_names + ([partition_name]
                                           if partition_name else [])

    def _make_body(k):
        def _body(*args):
            ins = list(args[:n_params])
            outs = list(args[n_params:])
            for _ in range(k):
                operands = ins + outs
                if partition_name is not None:
                    operands.append(bass2jax.partition_id_tensor())
                outs = list(bass2jax._bass_exec_p.bind(
                    *operands,
                    out_avals=tuple(out_avals),
                    in_names=tuple(all_in_names),
                    out_names=tuple(out_names),
                    lowering_input_output_aliases=(),
                    sim_require_finite=True,
                    sim_require_nnan=True,
                    nc=nc,
                ))
            return tuple(outs)
        return _body

    devices = jax.devices()[:n_cores]
    mesh = Mesh(np.asarray(devices), ("core",))
    n_outs = len(out_names)
    specs = (PartitionSpec("core"),) * (n_params + n_outs)
    o_specs = (PartitionSpec("core"),) * n_outs
    run1 = jax.jit(
        shard_map(_make_body(1), mesh=mesh, in_specs=specs,
                  out_specs=o_specs, check_rep=False),
        keep_unused=True)
    concat_in = [
        np.concatenate([np.asarray(in_maps[c][nm]) for c in range(n_cores)],
                       axis=0)
        for nm in in_names
    ]
    concat_zeros = [np.zeros((n_cores * z.shape[0], *z.shape[1:]), z.dtype)
                    for z in zero_outs]
    shard = NamedSharding(mesh, PartitionSpec("core"))
    dev_args = [jax.device_put(a, shard)
                for a in (*concat_in, *concat_zeros)]
    out_arrs = run1(*dev_args)
    jax.block_until_ready(out_arrs)

    best_ns = None
    if bench:
        # pipelined-launch throughput: back-to-back async launches overlap
        # the ~85ms axon round-trip; the marginal cost per launch is the
        # device execution (+ per-launch dispatch overhead). Report the
        # slope between a short and a long pipeline, which cancels the
        # fixed round-trip latency.
        from concurrent.futures import ThreadPoolExecutor
        M1, M2 = 8, CHAIN_K

        def _pipe(m):
            t0 = time.perf_counter_ns()
            rs = [run1(*dev_args) for _ in range(m)]
            jax.block_until_ready(rs)
            return time.perf_counter_ns() - t0

        def _pipe_t(m, nthreads=4):
            t0 = time.perf_counter_ns()
            with ThreadPoolExecutor(nthreads) as ex:
                rs = list(ex.map(lambda _: run1(*dev_args), range(m)))
            jax.block_until_ready(rs)
            return time.perf_counter_ns() - t0

        slopes, tslopes = [], []
        try:
            # sustained warmup (~2s) so device clocks settle
            for _ in range(3):
                _pipe(CHAIN_K)
            for _ in range(6):
                ta = _pipe(M1)
                tb = _pipe(M2)
                slopes.append((tb - ta) / (M2 - M1))
            for _ in range(2):
                ta = _pipe_t(M1)
                tb = _pipe_t(M2)
                tslopes.append((tb - ta) / (M2 - M1))
        except Exception as e:
            print(f"[bench] pipeline bench failed: {e}", flush=True)
        print(f"[bench] pipelined slope per launch: "
              f"{[int(s) for s in slopes]} ns; threaded: "
              f"{[int(s) for s in tslopes]} ns", flush=True)
        if slopes + tslopes:
            best_ns = max(1, int(round(min(slopes + tslopes))))
        else:
            # fallback: single blocking launch (includes the full RTT)
            t0 = time.perf_counter_ns()
            jax.block_until_ready(run1(*dev_args))
            best_ns = time.perf_counter_ns() - t0

    results = [
        {nm: np.asarray(out_arrs[i]).reshape(n_cores, *out_avals[i].shape)[c]
         for i, nm in enumerate(out_names)}
        for c in range(n_cores)
    ]
    return results, best_ns


def kernel(x, edge_index, W, att_src, att_dst, bias, Wl, bl):
    global LAST_EXEC_NS
    shared, percore, TW, assign = _host_prep(
        x, edge_index, W, att_src, att_dst, bias, Wl, bl)

    if TW not in _PROG_CACHE:
        _PROG_CACHE[TW] = build_program(TW)
    nc = _PROG_CACHE[TW]

    in_maps = [dict(shared, **percore[c]) for c in range(N_CORES)]

    if os.environ.get("BASS_GAT_SIM"):
        from concourse.bass_interp import CoreSim
        outs = []
        for c in range(int(os.environ.get("BASS_GAT_SIM_CORES", N_CORES))):
            sim = CoreSim(nc)
            for k, v in in_maps[c].items():
                sim.tensor(k)[:] = v
            sim.simulate()
            outs.append(np.array(sim.tensor("out")))
        while len(outs) < N_CORES:
            outs.append(np.zeros((NLOC, D), np.float32))
    else:
        bench = os.environ.get("BASS_GAT_BENCH", "1") != "0"
        results, best_ns = _run_pjrt(nc, in_maps, N_CORES, bench=bench)
        outs = [r["out"] for r in results]
        LAST_EXEC_NS = best_ns
    # unpermute: core c's slot s holds dst-block assign[s, c]
    full = np.zeros((NT, P, D), np.float32)
    for c in range(N_CORES):
        full[assign[:, c]] = np.asarray(outs[c]).reshape(BLOCKS, P, D)
    full = full.reshape(NT * P, D)[:N]
    return np.ascontiguousarray(full.astype(np.float32))
